# revision 2
# baseline (speedup 1.0000x reference)
"""MiniMoE Trainium2 kernel (expert-parallel, F-split across core pairs).

Problem (hardcoded): x [4, 2048, 1024] f32, router_w [1024, 4], router_b [4],
w1/w3 [4, 1024, 4096], w2 [4, 4096, 1024], top-2 of 4 experts, SwiGLU.

Strategy
--------
Host computes the (tiny) router + top-2 dispatch — this *is* the sharding
decision ("all-to-all token dispatch by top_indices"). Core pair (2e, 2e+1)
owns expert e: core 2e computes the F in [0, 2048) half of the SwiGLU FFN,
core 2e+1 the F in [2048, 4096) half, each over ALL tokens routed to expert
e. The two partial outputs sum to the expert output (h @ w2 is linear in h),
and the host scatter-adds them back with the renormalized gate weights.

On-device layout keeps features on partitions and tokens on the free axis
(so no transposes are needed between the two matmuls):
  hT[f, t]   = silu(w1.T @ xT) * (w3.T @ xT)      f on partitions
  outT[d, t] = w2.T @ hT                           d on partitions
All weights for the core's F-half stay resident in SBUF (~12 MB bf16);
tokens stream through in blocks of 512.
"""

import numpy as np
import ml_dtypes

import concourse.bass as bass
import concourse.bacc as bacc
import concourse.tile as tile
import concourse.mybir as mybir
from concourse.bass_utils import run_bass_kernel_spmd

B, S, D, F, E, TOPK = 4, 2048, 1024, 4096, 4, 2
N_CORES = 8
FH = F // 2          # F-half handled per core
P = 128              # SBUF partitions
ND = D // P          # 8 d-blocks
NF = FH // P         # 16 f-blocks per core
BF16 = mybir.dt.bfloat16
F32 = mybir.dt.float32

_NC_CACHE: dict[int, object] = {}


def _token_blocks(C: int) -> list[tuple[int, int]]:
    blocks = []
    t0 = 0
    while t0 < C:
        tb = min(512, C - t0)
        blocks.append((t0, tb))
        t0 += tb
    return blocks


def _build_nc(C: int):
    """Build + compile the SPMD per-core program for capacity C tokens."""
    nc = bacc.Bacc("TRN2", target_bir_lowering=False, debug=False,
                   num_devices=N_CORES)
    xT = nc.dram_tensor("xT", [D, C], BF16, kind="ExternalInput").ap()
    w1 = nc.dram_tensor("w1", [D, FH], BF16, kind="ExternalInput").ap()
    w3 = nc.dram_tensor("w3", [D, FH], BF16, kind="ExternalInput").ap()
    w2 = nc.dram_tensor("w2", [FH, D], BF16, kind="ExternalInput").ap()
    outT = nc.dram_tensor("outT", [D, C], F32, kind="ExternalOutput").ap()

    with tile.TileContext(nc) as tc:
        with (
            tc.tile_pool(name="wpool", bufs=1) as wpool,
            tc.tile_pool(name="xpool", bufs=2) as xpool,
            tc.tile_pool(name="hpool", bufs=2) as hpool,
            tc.tile_pool(name="tpool", bufs=3) as tpool,
            tc.tile_pool(name="opool", bufs=3) as opool,
            tc.tile_pool(name="ps1", bufs=2, space=bass.MemorySpace.PSUM) as ps1,
            tc.tile_pool(name="ps2", bufs=4, space=bass.MemorySpace.PSUM) as ps2,
        ):
            # Resident weights: partitions hold the contraction dim slice.
            w1_sb = wpool.tile([P, ND, FH], BF16, tag="w1")
            w3_sb = wpool.tile([P, ND, FH], BF16, tag="w3")
            w2_sb = wpool.tile([P, NF, D], BF16, tag="w2")
            nc.sync.dma_start(w1_sb[:], w1.rearrange("(n p) f -> p n f", p=P))
            nc.sync.dma_start(w3_sb[:], w3.rearrange("(n p) f -> p n f", p=P))
            nc.sync.dma_start(w2_sb[:], w2.rearrange("(n p) d -> p n d", p=P))

            xT_r = xT.rearrange("(n p) c -> p n c", p=P)

            for (t0, TB) in _token_blocks(C):
                xtb = xpool.tile([P, ND, TB], BF16, tag="xtb")
                nc.sync.dma_start(xtb[:], xT_r[:, :, t0:t0 + TB])

                hT = hpool.tile([P, NF, TB], BF16, tag="hT")
                for ft in range(NF):
                    p1 = ps1.tile([P, TB], F32, tag="p1")
                    p3 = ps1.tile([P, TB], F32, tag="p3")
                    for d in range(ND):
                        nc.tensor.matmul(
                            p1[:], w1_sb[:, d, ft * P:(ft + 1) * P],
                            xtb[:, d, :], start=(d == 0), stop=(d == ND - 1))
                    for d in range(ND):
                        nc.tensor.matmul(
                            p3[:], w3_sb[:, d, ft * P:(ft + 1) * P],
                            xtb[:, d, :], start=(d == 0), stop=(d == ND - 1))
                    sil = tpool.tile([P, TB], F32, tag="sil")
                    nc.scalar.activation(
                        sil[:], p1[:], mybir.ActivationFunctionType.Silu)
                    nc.vector.tensor_mul(hT[:, ft, :], sil[:], p3[:])

                for db in range(ND):
                    po = ps2.tile([P, TB], F32, tag="po")
                    for ft in range(NF):
                        nc.tensor.matmul(
                            po[:], w2_sb[:, ft, db * P:(db + 1) * P],
                            hT[:, ft, :], start=(ft == 0), stop=(ft == NF - 1))
                    ot = opool.tile([P, TB], F32, tag="ot")
                    nc.scalar.copy(ot[:], po[:])
                    nc.sync.dma_start(outT[db * P:(db + 1) * P, t0:t0 + TB], ot[:])

    nc.compile()
    return nc


def _route(x, router_w, router_b):
    """Host router: top-2 expert ids + renormalized gates (float64 math)."""
    T = x.shape[0] * x.shape[1]
    xf = x.reshape(T, D).astype(np.float64)
    logits = xf @ router_w.astype(np.float64) + router_b.astype(np.float64)
    order = np.argsort(-logits, axis=-1)          # [T, E] descending
    top_i = order[:, :TOPK]                        # [T, 2]
    top_l = np.take_along_axis(logits, top_i, axis=-1)
    top_l -= top_l.max(axis=-1, keepdims=True)
    ex = np.exp(top_l)
    gates = ex / ex.sum(axis=-1, keepdims=True)    # [T, 2] renormalized
    return top_i, gates


def prepare(x, router_w, router_b, w1, w3, w2):
    """Route on host, build per-core input maps. Returns (C, in_maps, meta)."""
    T = x.shape[0] * x.shape[1]
    xf = np.ascontiguousarray(x.reshape(T, D), dtype=np.float32)
    top_i, gates = _route(x, router_w, router_b)

    idx_per_e = []
    gate_per_e = []
    for e in range(E):
        mask = (top_i == e)
        rows = np.nonzero(mask.any(axis=-1))[0]
        g = np.where(mask[rows, 0], gates[rows, 0], gates[rows, 1])
        idx_per_e.append(rows)
        gate_per_e.append(g.astype(np.float32))

    n_max = max(len(r) for r in idx_per_e)
    C = ((n_max + P - 1) // P) * P

    in_maps = []
    for core in range(N_CORES):
        e, half = core // 2, core % 2
        fs = slice(half * FH, (half + 1) * FH)
        rows = idx_per_e[e]
        xg = np.zeros((C, D), np.float32)
        xg[:len(rows)] = xf[rows]
        in_maps.append({
            "xT": np.ascontiguousarray(xg.T).astype(ml_dtypes.bfloat16),
            "w1": np.ascontiguousarray(w1[e, :, fs]).astype(ml_dtypes.bfloat16),
            "w3": np.ascontiguousarray(w3[e, :, fs]).astype(ml_dtypes.bfloat16),
            "w2": np.ascontiguousarray(w2[e, fs, :]).astype(ml_dtypes.bfloat16),
        })
    meta = (T, idx_per_e, gate_per_e)
    return C, in_maps, meta


def combine(results, meta):
    """Gate-weighted scatter-add of the per-core partial expert outputs."""
    T, idx_per_e, gate_per_e = meta
    out = np.zeros((T, D), np.float32)
    for e in range(E):
        rows = idx_per_e[e]
        n = len(rows)
        part = (results[2 * e]["outT"].T[:n].astype(np.float32)
                + results[2 * e + 1]["outT"].T[:n].astype(np.float32))
        out[rows] += gate_per_e[e][:, None] * part
    return out.reshape(B, S, D)


def kernel(**inputs):
    x = np.asarray(inputs["x"], np.float32)
    router_w = np.asarray(inputs["router_w"], np.float32)
    router_b = np.asarray(inputs["router_b"], np.float32)
    w1 = np.asarray(inputs["w1"], np.float32)
    w3 = np.asarray(inputs["w3"], np.float32)
    w2 = np.asarray(inputs["w2"], np.float32)

    C, in_maps, meta = prepare(x, router_w, router_b, w1, w3, w2)
    if C not in _NC_CACHE:
        _NC_CACHE[C] = _build_nc(C)
    nc = _NC_CACHE[C]
    res = run_bass_kernel_spmd(nc, in_maps, list(range(N_CORES)))
    return combine(res.results, meta)


# revision 17
# speedup vs baseline: 58.4388x; 58.4388x over previous
"""MiniMoE Trainium2 kernel (expert-parallel, F-split across core pairs).

Problem (hardcoded): x [4, 2048, 1024] f32, router_w [1024, 4], router_b [4],
w1/w3 [4, 1024, 4096], w2 [4, 4096, 1024], top-2 of 4 experts, SwiGLU.

Strategy
--------
Host computes the (tiny) router + top-2 dispatch — this *is* the sharding
decision ("all-to-all token dispatch by top_indices"). Core pair (2e, 2e+1)
owns expert e: core 2e computes the F in [0, 2048) half of the SwiGLU FFN,
core 2e+1 the F in [2048, 4096) half, each over ALL tokens routed to expert
e. The two partial outputs sum to the expert output (h @ w2 is linear in h),
and the host scatter-adds them back with the renormalized gate weights.

On-device layout keeps features on partitions and tokens on the free axis
(so no transposes are needed between the two matmuls):
  hT[f, t]   = silu(w1.T @ xT) * (w3.T @ xT)      f on partitions
  outT[d, t] = w2.T @ hT                           d on partitions
All weights for the core's F-half stay resident in SBUF (~12 MB bf16);
tokens stream through in blocks of 512.
"""

import numpy as np
import ml_dtypes

import concourse.bass as bass
import concourse.bacc as bacc
import concourse.tile as tile
import concourse.mybir as mybir
from concourse.bass_utils import run_bass_kernel_spmd

B, S, D, F, E, TOPK = 4, 2048, 1024, 4096, 4, 2
N_CORES = 8
FH = F // 2          # F-half handled per core
P = 128              # SBUF partitions
ND = D // P          # 8 d-blocks
NF = FH // P         # 16 f-blocks per core
BF16 = mybir.dt.bfloat16
F32 = mybir.dt.float32

_NC_CACHE: dict[int, object] = {}


def _token_blocks(C: int) -> list[tuple[int, int]]:
    """Token blocks of 512, but split a short tail across the last two
    blocks (e.g. 512+128 -> 320+320): matmul N=320 pipelines against the
    128-cycle weight loads far better than N=128 does."""
    sizes = []
    left = C
    while left > 0:
        tb = min(512, left)
        sizes.append(tb)
        left -= tb
    if len(sizes) >= 2 and sizes[-1] < 512:
        pair = sizes[-2] + sizes[-1]
        hi = ((pair // 2 + 63) // 64) * 64
        sizes[-2:] = [hi, pair - hi]
    blocks, t0 = [], 0
    for tb in sizes:
        blocks.append((t0, tb))
        t0 += tb
    return blocks


def _build_nc(C: int, repeat: int = 1, ft_chunks: bool = True):
    """Build + compile the SPMD per-core program for capacity C tokens.

    repeat > 1 re-runs the whole token loop (timing harness use only —
    lets test.py fit out the fixed dispatch overhead via the slope).
    ft_chunks: load w1/w3 as 16 host-pre-tiled f-tile chunks (w1t/w3t
    inputs) instead of 8 d-row chunks of the plain [D, FH] layout."""
    nc = bacc.Bacc("TRN2", target_bir_lowering=False, debug=False,
                   num_devices=N_CORES)
    xT = nc.dram_tensor("xT", [D, C], BF16, kind="ExternalInput").ap()
    if ft_chunks:
        # Host-pre-tiled [NF, P, ND*P]: chunk ft is exactly the SBUF tile
        # for f-tile ft, so each chunk loads as one contiguous DMA.
        w1 = nc.dram_tensor("w1t", [NF, P, ND * P], BF16, kind="ExternalInput").ap()
        w3 = nc.dram_tensor("w3t", [NF, P, ND * P], BF16, kind="ExternalInput").ap()
    else:
        w1 = nc.dram_tensor("w1", [D, FH], BF16, kind="ExternalInput").ap()
        w3 = nc.dram_tensor("w3", [D, FH], BF16, kind="ExternalInput").ap()
    w2 = nc.dram_tensor("w2", [FH, D], BF16, kind="ExternalInput").ap()
    outT = nc.dram_tensor("outT", [D, C], F32, kind="ExternalOutput").ap()

    with tile.TileContext(nc) as tc:
        with (
            tc.tile_pool(name="wpool", bufs=1) as wpool,
            tc.tile_pool(name="xpool", bufs=2) as xpool,
            tc.tile_pool(name="hpool", bufs=2) as hpool,
            tc.tile_pool(name="tpool", bufs=3) as tpool,
            tc.tile_pool(name="opool", bufs=3) as opool,
            tc.tile_pool(name="ps1", bufs=2, space=bass.MemorySpace.PSUM) as ps1,
            tc.tile_pool(name="ps2", bufs=4, space=bass.MemorySpace.PSUM) as ps2,
        ):
            # Resident weights: partitions hold the contraction dim slice.
            # w1/w3 are chunked BY F-TILE (the phase-1 consumption order):
            # the first f-group only waits for ~0.5 MB instead of the whole
            # 16 MB, and demand then ramps at ~0.15 MB/us < DMA rate.
            w2_r = w2.rearrange("(n p) d -> n p d", p=P)
            w1_f, w3_f, w2_f = [], [], []
            if ft_chunks:
                for ft in range(NF):
                    t1 = wpool.tile([P, ND, P], BF16, tag=f"w1_{ft}")
                    t3 = wpool.tile([P, ND, P], BF16, tag=f"w3_{ft}")
                    nc.sync.dma_start(t1[:], w1[ft].rearrange("p (n c) -> p n c", c=P))
                    nc.sync.dma_start(t3[:], w3[ft].rearrange("p (n c) -> p n c", c=P))
                    w1_f.append(t1)
                    w3_f.append(t3)
            else:
                w1_r = w1.rearrange("(n p) f -> n p f", p=P)
                w3_r = w3.rearrange("(n p) f -> n p f", p=P)
                w1_d, w3_d = [], []
                for d in range(ND):
                    t1 = wpool.tile([P, FH], BF16, tag=f"w1d_{d}")
                    t3 = wpool.tile([P, FH], BF16, tag=f"w3d_{d}")
                    nc.sync.dma_start(t1[:], w1_r[d])
                    nc.sync.dma_start(t3[:], w3_r[d])
                    w1_d.append(t1)
                    w3_d.append(t3)

                class _DView:
                    def __init__(self, tiles):
                        self.tiles = tiles

                    def __getitem__(self, ft):
                        return _FtView(self.tiles, ft)

                class _FtView:
                    def __init__(self, tiles, ft):
                        self.tiles, self.ft = tiles, ft

                    def __getitem__(self, key):
                        _, d, _ = key
                        f = self.ft
                        return self.tiles[d][:, f * P:(f + 1) * P]

                w1_f = _DView(w1_d)
                w3_f = _DView(w3_d)
            for ft in range(NF):
                t2 = wpool.tile([P, D], BF16, tag=f"w2_{ft}")
                nc.sync.dma_start(t2[:], w2_r[ft])
                w2_f.append(t2)

            xT_r = xT.rearrange("(n p) c -> p n c", p=P)

            for (t0, TB) in _token_blocks(C) * repeat:
                xtb = xpool.tile([P, ND, TB], BF16, tag="xtb")
                nc.sync.dma_start(xtb[:], xT_r[:, :, t0:t0 + TB])

                hT = hpool.tile([P, NF, TB], BF16, tag="hT")
                for ft in range(NF):
                    p1 = ps1.tile([P, TB], F32, tag="p1")
                    p3 = ps1.tile([P, TB], F32, tag="p3")
                    for d in range(ND):
                        nc.tensor.matmul(
                            p1[:], w1_f[ft][:, d, :],
                            xtb[:, d, :], start=(d == 0), stop=(d == ND - 1))
                    for d in range(ND):
                        nc.tensor.matmul(
                            p3[:], w3_f[ft][:, d, :],
                            xtb[:, d, :], start=(d == 0), stop=(d == ND - 1))
                    sil = tpool.tile([P, TB], F32, tag="sil")
                    nc.scalar.activation(
                        sil[:], p1[:], mybir.ActivationFunctionType.Silu)
                    nc.vector.tensor_mul(hT[:, ft, :], sil[:], p3[:])

                for db in range(ND):
                    po = ps2.tile([P, TB], F32, tag="po")
                    for ft in range(NF):
                        nc.tensor.matmul(
                            po[:], w2_f[ft][:, db * P:(db + 1) * P],
                            hT[:, ft, :], start=(ft == 0), stop=(ft == NF - 1))
                    ot = opool.tile([P, TB], F32, tag="ot")
                    nc.scalar.copy(ot[:], po[:])
                    nc.sync.dma_start(outT[db * P:(db + 1) * P, t0:t0 + TB], ot[:])

    nc.compile()
    return nc


def _route(x, router_w, router_b):
    """Host router: top-2 expert ids + renormalized gates (float64 math)."""
    T = x.shape[0] * x.shape[1]
    xf = x.reshape(T, D).astype(np.float64)
    logits = xf @ router_w.astype(np.float64) + router_b.astype(np.float64)
    # stable sort: ties resolve to the lowest expert id, like jax.lax.top_k
    order = np.argsort(-logits, axis=-1, kind="stable")   # [T, E] descending
    top_i = order[:, :TOPK]                        # [T, 2]
    top_l = np.take_along_axis(logits, top_i, axis=-1)
    top_l -= top_l.max(axis=-1, keepdims=True)
    ex = np.exp(top_l)
    gates = ex / ex.sum(axis=-1, keepdims=True)    # [T, 2] renormalized
    return top_i, gates


def prepare(x, router_w, router_b, w1, w3, w2):
    """Route on host, build per-core input maps. Returns (C, in_maps, meta)."""
    T = x.shape[0] * x.shape[1]
    xf = np.ascontiguousarray(x.reshape(T, D), dtype=np.float32)
    top_i, gates = _route(x, router_w, router_b)

    idx_per_e = []
    gate_per_e = []
    for e in range(E):
        mask = (top_i == e)
        rows = np.nonzero(mask.any(axis=-1))[0]
        g = np.where(mask[rows, 0], gates[rows, 0], gates[rows, 1])
        idx_per_e.append(rows)
        gate_per_e.append(g.astype(np.float32))

    n_max = max(len(r) for r in idx_per_e)
    C = ((n_max + P - 1) // P) * P

    in_maps = []
    for core in range(N_CORES):
        e, half = core // 2, core % 2
        fs = slice(half * FH, (half + 1) * FH)
        rows = idx_per_e[e]
        xg = np.zeros((C, D), np.float32)
        xg[:len(rows)] = xf[rows]

        def tile_w(w):  # [D, FH] -> [NF, P, ND*P], chunk ft == SBUF tile ft
            return np.ascontiguousarray(
                w.reshape(ND, P, NF, P).transpose(2, 1, 0, 3).reshape(NF, P, ND * P))

        w1e = w1[e, :, fs].astype(ml_dtypes.bfloat16)
        w3e = w3[e, :, fs].astype(ml_dtypes.bfloat16)
        in_maps.append({
            "xT": np.ascontiguousarray(xg.T).astype(ml_dtypes.bfloat16),
            "w1": np.ascontiguousarray(w1e),
            "w3": np.ascontiguousarray(w3e),
            "w1t": tile_w(w1e),
            "w3t": tile_w(w3e),
            "w2": np.ascontiguousarray(w2[e, fs, :]).astype(ml_dtypes.bfloat16),
        })
    meta = (T, idx_per_e, gate_per_e)
    return C, in_maps, meta


def combine(results, meta):
    """Gate-weighted scatter-add of the per-core partial expert outputs."""
    T, idx_per_e, gate_per_e = meta
    out = np.zeros((T, D), np.float32)
    for e in range(E):
        rows = idx_per_e[e]
        n = len(rows)
        part = (results[2 * e]["outT"].T[:n].astype(np.float32)
                + results[2 * e + 1]["outT"].T[:n].astype(np.float32))
        out[rows] += gate_per_e[e][:, None] * part
    return out.reshape(B, S, D)


def kernel(**inputs):
    x = np.asarray(inputs["x"], np.float32)
    router_w = np.asarray(inputs["router_w"], np.float32)
    router_b = np.asarray(inputs["router_b"], np.float32)
    w1 = np.asarray(inputs["w1"], np.float32)
    w3 = np.asarray(inputs["w3"], np.float32)
    w2 = np.asarray(inputs["w2"], np.float32)

    C, in_maps, meta = prepare(x, router_w, router_b, w1, w3, w2)
    if C not in _NC_CACHE:
        _NC_CACHE[C] = _build_nc(C)
    nc = _NC_CACHE[C]
    res = run_bass_kernel_spmd(nc, in_maps, list(range(N_CORES)))
    return combine(res.results, meta)


# revision 20
# speedup vs baseline: 62.7783x; 1.0743x over previous
"""MiniMoE Trainium2 kernel (expert-parallel, F-split across core pairs).

Problem (hardcoded): x [4, 2048, 1024] f32, router_w [1024, 4], router_b [4],
w1/w3 [4, 1024, 4096], w2 [4, 4096, 1024], top-2 of 4 experts, SwiGLU.

Strategy
--------
Host computes the (tiny) router + top-2 dispatch — this *is* the sharding
decision ("all-to-all token dispatch by top_indices"). Core pair (2e, 2e+1)
owns expert e: core 2e computes the F in [0, 2048) half of the SwiGLU FFN,
core 2e+1 the F in [2048, 4096) half, each over ALL tokens routed to expert
e. The two partial outputs sum to the expert output (h @ w2 is linear in h),
and the host scatter-adds them back with the renormalized gate weights.

On-device layout keeps features on partitions and tokens on the free axis
(so no transposes are needed between the two matmuls):
  hT[f, t]   = silu(w1.T @ xT) * (w3.T @ xT)      f on partitions
  outT[d, t] = w2.T @ hT                           d on partitions
All weights for the core's F-half stay resident in SBUF (~12 MB bf16);
tokens stream through in blocks of 512.
"""

import numpy as np
import ml_dtypes

import concourse.bass as bass
import concourse.bacc as bacc
import concourse.tile as tile
import concourse.mybir as mybir
from concourse.bass_utils import run_bass_kernel_spmd

B, S, D, F, E, TOPK = 4, 2048, 1024, 4096, 4, 2
N_CORES = 8
FH = F // 2          # F-half handled per core
P = 128              # SBUF partitions
ND = D // P          # 8 d-blocks
NF = FH // P         # 16 f-blocks per core
BF16 = mybir.dt.bfloat16
F32 = mybir.dt.float32

_NC_CACHE: dict[int, object] = {}


def _token_blocks(C: int) -> list[tuple[int, int]]:
    """Token blocks of 512, but split a short tail across the last two
    blocks (e.g. 512+128 -> 320+320): matmul N=320 pipelines against the
    128-cycle weight loads far better than N=128 does."""
    sizes = []
    left = C
    while left > 0:
        tb = min(512, left)
        sizes.append(tb)
        left -= tb
    if len(sizes) >= 2 and sizes[-1] < 512:
        pair = sizes[-2] + sizes[-1]
        hi = ((pair // 2 + 63) // 64) * 64
        sizes[-2:] = [hi, pair - hi]
    blocks, t0 = [], 0
    for tb in sizes:
        blocks.append((t0, tb))
        t0 += tb
    return blocks


def _build_nc(C: int, repeat: int = 1, ft_chunks: bool = True):
    """Build + compile the SPMD per-core program for capacity C tokens.

    repeat > 1 re-runs the whole token loop (timing harness use only —
    lets test.py fit out the fixed dispatch overhead via the slope).
    ft_chunks: load w1/w3 as 16 host-pre-tiled f-tile chunks (w1t/w3t
    inputs) instead of 8 d-row chunks of the plain [D, FH] layout."""
    nc = bacc.Bacc("TRN2", target_bir_lowering=False, debug=False,
                   num_devices=N_CORES)
    xT = nc.dram_tensor("xT", [D, C], BF16, kind="ExternalInput").ap()
    if ft_chunks:
        # Host-pre-tiled [NF, P, ND*P]: chunk ft is exactly the SBUF tile
        # for f-tile ft, so each chunk loads as one contiguous DMA.
        w1 = nc.dram_tensor("w1t", [NF, P, ND * P], BF16, kind="ExternalInput").ap()
        w3 = nc.dram_tensor("w3t", [NF, P, ND * P], BF16, kind="ExternalInput").ap()
    else:
        w1 = nc.dram_tensor("w1", [D, FH], BF16, kind="ExternalInput").ap()
        w3 = nc.dram_tensor("w3", [D, FH], BF16, kind="ExternalInput").ap()
    w2 = nc.dram_tensor("w2", [FH, D], BF16, kind="ExternalInput").ap()
    outT = nc.dram_tensor("outT", [D, C], F32, kind="ExternalOutput").ap()

    with tile.TileContext(nc) as tc:
        with (
            tc.tile_pool(name="wpool", bufs=1) as wpool,
            tc.tile_pool(name="xpool", bufs=2) as xpool,
            tc.tile_pool(name="hpool", bufs=2) as hpool,
            tc.tile_pool(name="tpool", bufs=3) as tpool,
            tc.tile_pool(name="opool", bufs=3) as opool,
            tc.tile_pool(name="ps1", bufs=2, space=bass.MemorySpace.PSUM) as ps1,
            tc.tile_pool(name="ps2", bufs=4, space=bass.MemorySpace.PSUM) as ps2,
        ):
            # Resident weights: partitions hold the contraction dim slice.
            # w1/w3 are chunked BY F-TILE (the phase-1 consumption order):
            # the first f-group only waits for ~0.5 MB instead of the whole
            # 16 MB, and demand then ramps at ~0.15 MB/us < DMA rate.
            w2_r = w2.rearrange("(n p) d -> n p d", p=P)
            blocks = _token_blocks(C) * repeat
            xT_r = xT.rearrange("(n p) c -> p n c", p=P)

            # The very first PE dependency is x for block 0 — issue its DMA
            # ahead of the 24 MB weight stream so the queue order matches
            # consumption order (kills a ~37 us PE stall at startup).
            t0_first, TB_first = blocks[0]
            xtb0 = xpool.tile([P, ND, TB_first], BF16, tag="xtb")
            nc.sync.dma_start(xtb0[:], xT_r[:, :, t0_first:t0_first + TB_first])

            w1_f, w3_f, w2_f = [], [], []
            if ft_chunks:
                for ft in range(NF):
                    t1 = wpool.tile([P, ND, P], BF16, tag=f"w1_{ft}")
                    t3 = wpool.tile([P, ND, P], BF16, tag=f"w3_{ft}")
                    nc.sync.dma_start(t1[:], w1[ft].rearrange("p (n c) -> p n c", c=P))
                    nc.sync.dma_start(t3[:], w3[ft].rearrange("p (n c) -> p n c", c=P))
                    w1_f.append(t1)
                    w3_f.append(t3)
            else:
                w1_r = w1.rearrange("(n p) f -> n p f", p=P)
                w3_r = w3.rearrange("(n p) f -> n p f", p=P)
                w1_d, w3_d = [], []
                for d in range(ND):
                    t1 = wpool.tile([P, FH], BF16, tag=f"w1d_{d}")
                    t3 = wpool.tile([P, FH], BF16, tag=f"w3d_{d}")
                    nc.sync.dma_start(t1[:], w1_r[d])
                    nc.sync.dma_start(t3[:], w3_r[d])
                    w1_d.append(t1)
                    w3_d.append(t3)

                class _DView:
                    def __init__(self, tiles):
                        self.tiles = tiles

                    def __getitem__(self, ft):
                        return _FtView(self.tiles, ft)

                class _FtView:
                    def __init__(self, tiles, ft):
                        self.tiles, self.ft = tiles, ft

                    def __getitem__(self, key):
                        _, d, _ = key
                        f = self.ft
                        return self.tiles[d][:, f * P:(f + 1) * P]

                w1_f = _DView(w1_d)
                w3_f = _DView(w3_d)
            for ft in range(NF):
                t2 = wpool.tile([P, D], BF16, tag=f"w2_{ft}")
                nc.sync.dma_start(t2[:], w2_r[ft])
                w2_f.append(t2)

            for bi, (t0, TB) in enumerate(blocks):
                if bi == 0:
                    xtb = xtb0
                else:
                    xtb = xpool.tile([P, ND, TB], BF16, tag="xtb")
                    nc.sync.dma_start(xtb[:], xT_r[:, :, t0:t0 + TB])

                hT = hpool.tile([P, NF, TB], BF16, tag="hT")
                for ft in range(NF):
                    p1 = ps1.tile([P, TB], F32, tag="p1")
                    p3 = ps1.tile([P, TB], F32, tag="p3")
                    for d in range(ND):
                        nc.tensor.matmul(
                            p1[:], w1_f[ft][:, d, :],
                            xtb[:, d, :], start=(d == 0), stop=(d == ND - 1))
                    for d in range(ND):
                        nc.tensor.matmul(
                            p3[:], w3_f[ft][:, d, :],
                            xtb[:, d, :], start=(d == 0), stop=(d == ND - 1))
                    sil = tpool.tile([P, TB], F32, tag="sil")
                    nc.scalar.activation(
                        sil[:], p1[:], mybir.ActivationFunctionType.Silu)
                    nc.vector.tensor_mul(hT[:, ft, :], sil[:], p3[:])

                for db in range(ND):
                    po = ps2.tile([P, TB], F32, tag="po")
                    for ft in range(NF):
                        nc.tensor.matmul(
                            po[:], w2_f[ft][:, db * P:(db + 1) * P],
                            hT[:, ft, :], start=(ft == 0), stop=(ft == NF - 1))
                    ot = opool.tile([P, TB], F32, tag="ot")
                    nc.scalar.copy(ot[:], po[:])
                    nc.sync.dma_start(outT[db * P:(db + 1) * P, t0:t0 + TB], ot[:])

    nc.compile()
    return nc


def _route(x, router_w, router_b):
    """Host router: top-2 expert ids + renormalized gates (float64 math)."""
    T = x.shape[0] * x.shape[1]
    xf = x.reshape(T, D).astype(np.float64)
    logits = xf @ router_w.astype(np.float64) + router_b.astype(np.float64)
    # stable sort: ties resolve to the lowest expert id, like jax.lax.top_k
    order = np.argsort(-logits, axis=-1, kind="stable")   # [T, E] descending
    top_i = order[:, :TOPK]                        # [T, 2]
    top_l = np.take_along_axis(logits, top_i, axis=-1)
    top_l -= top_l.max(axis=-1, keepdims=True)
    ex = np.exp(top_l)
    gates = ex / ex.sum(axis=-1, keepdims=True)    # [T, 2] renormalized
    return top_i, gates


def prepare(x, router_w, router_b, w1, w3, w2):
    """Route on host, build per-core input maps. Returns (C, in_maps, meta)."""
    T = x.shape[0] * x.shape[1]
    xf = np.ascontiguousarray(x.reshape(T, D), dtype=np.float32)
    top_i, gates = _route(x, router_w, router_b)

    idx_per_e = []
    gate_per_e = []
    for e in range(E):
        mask = (top_i == e)
        rows = np.nonzero(mask.any(axis=-1))[0]
        g = np.where(mask[rows, 0], gates[rows, 0], gates[rows, 1])
        idx_per_e.append(rows)
        gate_per_e.append(g.astype(np.float32))

    n_max = max(len(r) for r in idx_per_e)
    C = ((n_max + P - 1) // P) * P

    in_maps = []
    for core in range(N_CORES):
        e, half = core // 2, core % 2
        fs = slice(half * FH, (half + 1) * FH)
        rows = idx_per_e[e]
        xg = np.zeros((C, D), np.float32)
        xg[:len(rows)] = xf[rows]

        def tile_w(w):  # [D, FH] -> [NF, P, ND*P], chunk ft == SBUF tile ft
            return np.ascontiguousarray(
                w.reshape(ND, P, NF, P).transpose(2, 1, 0, 3).reshape(NF, P, ND * P))

        w1e = w1[e, :, fs].astype(ml_dtypes.bfloat16)
        w3e = w3[e, :, fs].astype(ml_dtypes.bfloat16)
        in_maps.append({
            "xT": np.ascontiguousarray(xg.T).astype(ml_dtypes.bfloat16),
            "w1": np.ascontiguousarray(w1e),
            "w3": np.ascontiguousarray(w3e),
            "w1t": tile_w(w1e),
            "w3t": tile_w(w3e),
            "w2": np.ascontiguousarray(w2[e, fs, :]).astype(ml_dtypes.bfloat16),
        })
    meta = (T, idx_per_e, gate_per_e)
    return C, in_maps, meta


def combine(results, meta):
    """Gate-weighted scatter-add of the per-core partial expert outputs."""
    T, idx_per_e, gate_per_e = meta
    out = np.zeros((T, D), np.float32)
    for e in range(E):
        rows = idx_per_e[e]
        n = len(rows)
        part = (results[2 * e]["outT"].T[:n].astype(np.float32)
                + results[2 * e + 1]["outT"].T[:n].astype(np.float32))
        out[rows] += gate_per_e[e][:, None] * part
    return out.reshape(B, S, D)


def kernel(**inputs):
    x = np.asarray(inputs["x"], np.float32)
    router_w = np.asarray(inputs["router_w"], np.float32)
    router_b = np.asarray(inputs["router_b"], np.float32)
    w1 = np.asarray(inputs["w1"], np.float32)
    w3 = np.asarray(inputs["w3"], np.float32)
    w2 = np.asarray(inputs["w2"], np.float32)

    C, in_maps, meta = prepare(x, router_w, router_b, w1, w3, w2)
    if C not in _NC_CACHE:
        _NC_CACHE[C] = _build_nc(C)
    nc = _NC_CACHE[C]
    # prepare() emits both weight layouts (for A/B builds); pass the program
    # exactly its declared inputs — the native runner rejects extra keys.
    needed = {
        a.memorylocations[0].name
        for a in nc.m.functions[0].allocations
        if isinstance(a, mybir.MemoryLocationSet) and a.kind == "ExternalInput"
    }
    in_maps = [{k: v for k, v in m.items() if k in needed} for m in in_maps]
    res = run_bass_kernel_spmd(nc, in_maps, list(range(N_CORES)))
    return combine(res.results, meta)
